# revision 6
# baseline (speedup 1.0000x reference)
"""Trainium2 Bass kernel for nn_DilatedConv (dense_cnn).

Math: the torch in-place dilated-conv loop is the affine recurrence
    s[t+1] = A @ s[t] + c[t],   A = weight[:, :, 0],  c[t] = W1 @ x[:, :, t + n_dil]
over n_steps = 7935 transitions, s[0] = x[:, :, 0]; outputs overwrite
x[:, :, 1:7936].  rho(A) ~ 0.74, so ||A^32|| ~ 4e-4 and ||A^64|| ~ 2e-8:
long-range coupling is negligible and the global scan collapses to local
block scans with a one-block-lookback seam.

Parallelization: data-parallel over batch (16 -> 2 per core on 8 cores).
Per core, blocked scan over T = 7936 transitions:
  up1:   l1[g]  = sum_{j<8} (A^(7-j) W1) x[8g+j+256]      (8-block sums,
         W1 folded into the stacked powers -> x is read directly)
  up2:   l2[h]  = sum_{r<4} A^(8(3-r)) l1[4h+r]           (32-block sums)
  seam:  s[32h] = l2[h-1] + A^32 l2[h-2]  (+A^64 ... truncated, ~2e-8)
  down2: s[8g] for all g via 3 serial A^8 steps
  down1: s[8g+i] = A s[8g+i-1] + W1 x[8g+i+255]  (7 serial steps, W1 term
         fused into the same PSUM accumulation -> no separate c phase)
All matmuls keep a moving free dim >= 256 (f32r runs at bf16 rate there),
by interleaving the two local batch elements into one 2D access pattern.
down1 writes s[v] into the x-resident SBUF slot holding the just-consumed
x column (col v-1), so the final output DMA is a contiguous copy from the
x tile.  x is streamed in 7 units and output is streamed out per unit,
overlapping DMA with compute on both ends.
"""

import numpy as np

# ---------------- problem constants (hardcoded per spec) ----------------
B_FULL = 16
C = 256
N = 8192
N_DIL = 256
N_FILT = 2
N_CORES = 8
B_LOC = B_FULL // N_CORES          # 2

FILTER = (N_FILT - 1) * N_DIL + 1  # 257
N_STEPS = N - FILTER               # 7935
T = N - N_DIL                      # 7936 transitions incl. final unused one
G = T // 8                         # 992 8-blocks per batch element
H = G // 4                         # 248 32-blocks per batch element

# down1/up1 units: g-ranges (start, width); all widths >= 128 so the
# b-interleaved moving dim is >= 256 columns.
UNITS = [(0, 128), (128, 128), (256, 128), (384, 128),
         (512, 128), (640, 128), (768, 224)]
# level-2 chunks: h-ranges; second chunk overlaps the first by 8 h so both
# have 128 h (256 columns b-interleaved). Overlap recomputes identical
# values (benign WAW).
L2CHUNKS = [(0, 128), (120, 128)]

NT = 56  # weight-pack tiles


def _wi(kind, a, kc, mc):
    if kind == "BJ":
        return 4 * a + 2 * kc + mc          # j = 0..7  -> 0..31
    if kind == "A8P":
        return 32 + 4 * a + 2 * kc + mc     # r = 0..2  -> 32..43
    if kind == "A32":
        return 44 + 2 * kc + mc
    if kind == "A1":
        return 48 + 2 * kc + mc
    if kind == "W1":
        return 52 + 2 * kc + mc
    raise KeyError(kind)


def _host_pack(weight_f32):
    """Pack all lhsT 128x128 tiles into one (NT,128,128) float32 array.

    matmul(out, lhsT, rhs) computes lhsT.T @ rhs, so for out = Mat @ v the
    (kc, mc) tile is Mat.T[128kc:128(kc+1), 128mc:128(mc+1)].
    """
    A = weight_f32[:, :, 0].astype(np.float64)
    W1 = weight_f32[:, :, 1].astype(np.float64)

    def tiles(mat):
        mt = mat.T.astype(np.float32)
        return [mt[128 * kc:128 * (kc + 1), 128 * mc:128 * (mc + 1)]
                for kc in range(2) for mc in range(2)]

    Apow = [np.linalg.matrix_power(A, p) for p in range(9)]
    pack = []
    for j in range(8):
        pack += tiles(Apow[7 - j] @ W1)                    # BJ
    for r in range(3):
        pack += tiles(np.linalg.matrix_power(A, 8 * (3 - r)))  # A8P: A24,A16,A8
    pack += tiles(np.linalg.matrix_power(A, 32))           # A32
    pack += tiles(A)                                       # A1
    pack += tiles(W1)                                      # W1
    assert len(pack) == NT
    return np.stack(pack, axis=0)


def _build_program():
    """Build + bacc-compile the per-core Bass program. Returns nc."""
    import concourse.bacc as bacc
    import concourse.tile as tile
    from concourse import mybir

    f32r = mybir.dt.float32r
    f32 = mybir.dt.float32

    nc = bacc.Bacc("TRN2", target_bir_lowering=False, debug=False,
                   num_devices=N_CORES)
    x_in = nc.dram_tensor("x", [B_LOC, C, N], f32r, kind="ExternalInput").ap()
    wp_in = nc.dram_tensor("wpack", [NT, 128, 128], f32r,
                           kind="ExternalInput").ap()
    out = nc.dram_tensor("out", [B_LOC, C, N], f32r, kind="ExternalOutput").ap()

    with tile.TileContext(nc) as tc:
        import contextlib
        with contextlib.ExitStack() as ctx:
            wpool = ctx.enter_context(tc.tile_pool(name="wpool", bufs=1))
            xpool = ctx.enter_context(tc.tile_pool(name="xpool", bufs=1))
            spool = ctx.enter_context(tc.tile_pool(name="spool", bufs=1))
            pspool = ctx.enter_context(tc.tile_pool(name="ps", bufs=8,
                                                    space="PSUM"))

            # ---- persistent tiles ----
            wpk = wpool.tile([128, NT * 128], f32r, tag="wpk", name="wpk")
            wt = lambda kind, a, kc, mc: wpk[:, 128 * _wi(kind, a, kc, mc):
                                             128 * (_wi(kind, a, kc, mc) + 1)]
            # one extra leading column per batch element: s[8g+i] is written
            # into the slot one LEFT of the x column the step consumes, so
            # step 1 of block g=0 needs a column before x[256].
            TC = T + 1
            xres = [xpool.tile([128, B_LOC * TC], f32r, tag=f"x{kc}",
                               name=f"x{kc}") for kc in range(2)]
            l1 = [spool.tile([128, B_LOC * G], f32r, tag=f"l1_{mc}",
                             name=f"l1_{mc}") for mc in range(2)]
            u1 = [spool.tile([128, B_LOC * G], f32r, tag=f"u1_{mc}",
                             name=f"u1_{mc}") for mc in range(2)]
            zz = [spool.tile([128, B_LOC * (H + 2)], f32r, tag=f"zz_{mc}",
                             name=f"zz_{mc}") for mc in range(2)]

            xr3 = [xres[kc].rearrange("p (b t) -> p b t", b=B_LOC)
                   for kc in range(2)]  # col 1+t holds x[:, 256+t]; col v-1 ends up s[v]
            l13 = [l1[mc].rearrange("p (b g) -> p b g", b=B_LOC)
                   for mc in range(2)]
            u13 = [u1[mc].rearrange("p (b g) -> p b g", b=B_LOC)
                   for mc in range(2)]
            zz3 = [zz[mc].rearrange("p (b k) -> p b k", b=B_LOC)
                   for mc in range(2)]

            def st(ap, start, count, step):
                # strided slice with exact end bound (strict AP indexing)
                return ap[:, :, start:start + step * (count - 1) + 1:step]

            # round-robin PSUM->SBUF copy engines (only DVE + ACT see PSUM)
            _cp = [0]

            def copy_ps(dst, src):
                if _cp[0] % 2 == 0:
                    nc.vector.tensor_copy(dst, src)
                else:
                    nc.scalar.copy(dst, src)
                _cp[0] += 1

            # ---- phase 0: DMAs ----
            # BJ tiles first so up1 can start as soon as x unit 0 lands.
            nc.sync.dma_start(
                wpk[:, 0:32 * 128].rearrange("p (t f) -> p t f", t=32),
                wp_in[0:32].rearrange("t p f -> p t f"))
            for mc in range(2):
                for b in range(B_LOC):
                    nc.sync.dma_start(zz3[mc][:, b, 1:2],
                                      x_in[b, 128 * mc:128 * (mc + 1), 0:1])
                    # memset on f32r is invalid ISA; zero col 0 as s0 * 0
                    nc.vector.tensor_scalar_mul(zz3[mc][:, b, 0:1],
                                                zz3[mc][:, b, 1:2], 0.0)
                    nc.sync.dma_start(out[b, 128 * mc:128 * (mc + 1), 0:1],
                                      zz3[mc][:, b, 1:2])

            def dma_x_unit(u):
                g0, gw = UNITS[u]
                t0, tw = 8 * g0, 8 * gw
                for kc in range(2):
                    nc.sync.dma_start(
                        xr3[kc][:, :, 1 + t0:1 + t0 + tw],
                        x_in[:, 128 * kc:128 * (kc + 1),
                             N_DIL + t0:N_DIL + t0 + tw]
                        .rearrange("b p t -> p b t"))

            dma_x_unit(0)
            nc.sync.dma_start(
                wpk[:, 32 * 128:].rearrange("p (t f) -> p t f", t=NT - 32),
                wp_in[32:].rearrange("t p f -> p t f"))
            for u in range(1, len(UNITS)):
                dma_x_unit(u)
            # untouched tail: straight DRAM->DRAM
            nc.sync.dma_start(out[:, :, T:N], x_in[:, :, T:N])

            # ---- up1(u): l1[g] = sum_j (A^(7-j) W1) x[8g+j+256] ----
            def up1(u):
                g0, gw = UNITS[u]
                for mc in range(2):
                    ps = pspool.tile([128, B_LOC * gw], f32, tag="ps",
                                     name="ps")
                    for j in range(8):
                        for kc in range(2):
                            rhs = st(xr3[kc], 1 + 8 * g0 + j, gw, 8)
                            nc.tensor.matmul(ps[:], wt("BJ", j, kc, mc), rhs,
                                             start=(j == 0 and kc == 0),
                                             stop=(j == 7 and kc == 1))
                    copy_ps(l13[mc][:, :, g0:g0 + gw], ps[:])

            # ---- level 2 chunk: 32-block sums, seam, 3-step down scan ----
            def level2(c):
                h0, hw = L2CHUNKS[c]
                # up2: l2[h] -> zz[h+2]; the r=3 (identity) term rides the
                # PSUM->SBUF transfer as a DVE add.
                for mc in range(2):
                    ps = pspool.tile([128, B_LOC * hw], f32, tag="ps",
                                     name="ps")
                    for r in range(3):
                        for kc in range(2):
                            rhs = st(l13[kc], 4 * h0 + r, hw, 4)
                            nc.tensor.matmul(ps[:], wt("A8P", r, kc, mc), rhs,
                                             start=(r == 0 and kc == 0),
                                             stop=(r == 2 and kc == 1))
                    nc.vector.tensor_add(
                        zz3[mc][:, :, h0 + 2:h0 + 2 + hw], ps[:],
                        st(l13[mc], 4 * h0 + 3, hw, 4))
                # seam: u1[4h] = s[32h] = zz[h+1] + A32 zz[h]
                for mc in range(2):
                    ps = pspool.tile([128, B_LOC * hw], f32, tag="ps",
                                     name="ps")
                    for kc in range(2):
                        nc.tensor.matmul(ps[:], wt("A32", 0, kc, mc),
                                         zz3[kc][:, :, h0:h0 + hw],
                                         start=(kc == 0), stop=(kc == 1))
                    nc.vector.tensor_add(
                        st(u13[mc], 4 * h0, hw, 4), ps[:],
                        zz3[mc][:, :, h0 + 1:h0 + 1 + hw])
                # down2: u1[4h+r] = A8 u1[4h+r-1] + l1[4h+r-1]
                for r in range(1, 4):
                    for mc in range(2):
                        ps = pspool.tile([128, B_LOC * hw], f32, tag="ps",
                                         name="ps")
                        for kc in range(2):
                            rhs = st(u13[kc], 4 * h0 + r - 1, hw, 4)
                            nc.tensor.matmul(ps[:], wt("A8P", 2, kc, mc), rhs,
                                             start=(kc == 0), stop=(kc == 1))
                        nc.vector.tensor_add(
                            st(u13[mc], 4 * h0 + r, hw, 4), ps[:],
                            st(l13[mc], 4 * h0 + r - 1, hw, 4))

            # ---- down1(u): s[8g+i] = A s[8g+i-1] + W1 x[8g+i+255];
            # s[v] overwrites xres col v-1 (the just-consumed x slot), so the
            # output DMA is a contiguous slice of xres. ----
            def down1(u):
                g0, gw = UNITS[u]
                for i in range(1, 8):
                    for mc in range(2):
                        ps = pspool.tile([128, B_LOC * gw], f32, tag="ps",
                                         name="ps")
                        for kc in range(2):
                            # W1 term reads x t=8g+i-1 at col 8g+i
                            nc.tensor.matmul(
                                ps[:], wt("W1", 0, kc, mc),
                                st(xr3[kc], 8 * g0 + i, gw, 8),
                                start=(kc == 0), stop=False)
                        for kc in range(2):
                            # A term reads s[8g+i-1] at col 8g+i-2
                            if i == 1:
                                rhs = u13[kc][:, :, g0:g0 + gw]
                            else:
                                rhs = st(xr3[kc], 8 * g0 + i - 2, gw, 8)
                            nc.tensor.matmul(ps[:], wt("A1", 0, kc, mc), rhs,
                                             start=False, stop=(kc == 1))
                        # s[8g+i] -> col 8g+i-1: the x slot the PREVIOUS step
                        # consumed; never a read operand of this step.
                        copy_ps(st(xr3[mc], 8 * g0 + i - 1, gw, 8), ps[:])
                # residues: s[8g] = u1[g] -> col 8g-1, dead after step 7's
                # W1 read of the preceding block (this/previous unit).
                gr0 = max(g0, 1)
                for mc in range(2):
                    nc.gpsimd.tensor_copy(
                        st(xr3[mc], 8 * gr0 - 1, g0 + gw - gr0, 8),
                        u13[mc][:, :, gr0:g0 + gw])
                # this unit's out cols are final: stream them to DRAM
                v0 = max(8 * g0, 1)
                v1 = 8 * (g0 + gw)
                for mc in range(2):
                    nc.sync.dma_start(
                        out[:, 128 * mc:128 * (mc + 1), v0:v1]
                        .rearrange("b p t -> p b t"),
                        xr3[mc][:, :, v0 - 1:v1 - 1])

            # ---- pipeline ----
            up1(0)
            up1(1)
            up1(2)
            up1(3)
            level2(0)
            down1(0)
            up1(4)
            down1(1)
            up1(5)
            down1(2)
            up1(6)
            down1(3)
            level2(1)
            down1(4)
            down1(5)
            down1(6)

    nc.compile()
    return nc


_CACHE = {}


def _get_program():
    if "nc" not in _CACHE:
        _CACHE["nc"] = _build_program()
    return _CACHE["nc"]


LAST_RESULTS = None  # test harness reads exec_time_ns off this


def kernel(x, weight, n_dil):
    import os
    from concourse.bass_utils import run_bass_kernel_spmd
    global LAST_RESULTS

    x = np.asarray(x)
    weight = np.asarray(weight)
    assert int(n_dil) == N_DIL and x.shape == (B_FULL, C, N)
    nc = _get_program()
    wpack = _host_pack(weight.astype(np.float32))

    xs = x.astype(np.float32).reshape(N_CORES, B_LOC, C, N)
    in_maps = [{"x": xs[i], "wpack": wpack} for i in range(N_CORES)]
    trace = bool(os.environ.get("KERNEL_TRACE"))
    res = run_bass_kernel_spmd(nc, in_maps, list(range(N_CORES)), trace=trace)
    LAST_RESULTS = res
    out = np.concatenate([res.results[i]["out"] for i in range(N_CORES)],
                         axis=0)
    return out.astype(x.dtype, copy=False)


# revision 20
# speedup vs baseline: 1.6091x; 1.6091x over previous
"""Trainium2 Bass kernel for nn_DilatedConv (dense_cnn).

Math: the torch in-place dilated-conv loop is the affine recurrence
    s[t+1] = A @ s[t] + c[t],   A = weight[:, :, 0],  c[t] = W1 @ x[:, :, t + n_dil]
over n_steps = 7935 transitions, s[0] = x[:, :, 0]; outputs overwrite
x[:, :, 1:7936].  rho(A) ~ 0.74, so ||A^32|| ~ 4e-4 and ||A^64|| ~ 2e-8:
long-range coupling is negligible and the global scan collapses to local
block scans with a one-block-lookback seam.

Parallelization: data-parallel over batch (16 -> 2 per core on 8 cores).
Per core, blocked scan over T = 7936 transitions:
  up1:   l1[g]  = sum_{j<8} (A^(7-j) W1) x[8g+j+256]      (8-block sums,
         W1 folded into the stacked powers -> x is read directly)
  up2:   l2[h]  = sum_{r<4} A^(8(3-r)) l1[4h+r]           (32-block sums)
  seam:  s[32h] = l2[h-1] + A^32 l2[h-2]  (+A^64 ... truncated, ~2e-8)
  down2: s[8g] for all g via 3 serial A^8 steps
  down1: s[8g+i] = A s[8g+i-1] + W1 x[8g+i+255]  (7 serial steps, W1 term
         fused into the same PSUM accumulation -> no separate c phase and
         every PSUM->SBUF transfer is a pure copy split across DVE/ACT)

Performance structure (tuned against the CoreSim cost model):
 - All big matmuls keep a moving free dim >= 256 (f32r then runs at bf16
   rate, 1 cycle/column). up1/down1 work on per-batch-element units of
   [256, 480, 256] 8-blocks; level 2 interleaves the two batch elements
   (N = 2*hw) since its phases are tiny.
 - Per-(b,kc,unit) DMAs keep every transfer's byte interval local, so the
   Tile framework's interval-granular dependency tracking never creates
   false cross-unit serialization (b-interleaved layouts and b-merged
   DMAs both lose ~15-30us to false WAR/WAW edges).
 - down1 writes s[v] into xres col v (the x slot consumed by the PREVIOUS
   step, never an operand of the current step -- in-step aliasing of the
   W1 read is what broke the naive in-place scheme). Two leading columns
   per b hold s0/s1, so each unit's output DMA is one contiguous slice
   covering out col 0 as well.
 - All x-in DMAs are emitted (= queued) densely up front, output units
   drain behind them; out cols stream to DRAM per unit while later units
   compute. The untouched tail out[:, :, 7936:] is written from the
   resident x tile (WAR-ordered before down1 unit 2 overwrites it).
"""

import numpy as np

# ---------------- problem constants (hardcoded per spec) ----------------
B_FULL = 16
C = 256
N = 8192
N_DIL = 256
N_FILT = 2
N_CORES = 8
B_LOC = B_FULL // N_CORES          # 2

FILTER = (N_FILT - 1) * N_DIL + 1  # 257
N_STEPS = N - FILTER               # 7935
T = N - N_DIL                      # 7936 transitions incl. final unused one
G = T // 8                         # 992 8-blocks per batch element
H = G // 4                         # 248 32-blocks per batch element

# down1/up1 units: per-batch-element g-ranges (start, width), all >= 256 so
# every matmul moving dim is >= 256 columns (f32r full rate) WITHOUT
# interleaving the two batch elements (which would make every access span
# the whole tile and defeat interval-granular dependency tracking).
# First unit small (output stream starts after only ~4MB of x in), big unit
# in the middle, small last unit (short final-DMA tail).
_U = [256, 480, 256]
assert sum(_U) == G and all(u >= 256 for u in _U)
UNITS = [(sum(_U[:i]), _U[i]) for i in range(len(_U))]
# level-2 chunks: h-ranges, b-interleaved (N = 2*hw). Chunk k covers unit k
# exactly; chunks 1,2 overlap their predecessor (identical values, benign
# WAW) to keep width >= 128. Chunk 0 is N=128 (f32r slow rate) but tiny --
# it buys a much earlier start of the output stream.
L2CHUNKS = [(0, _U[0] // 4),
            ((_U[0] + _U[1]) // 4 - 128, 128),
            (G // 4 - 128, 128)]

NT = 56  # weight-pack tiles


def _wi(kind, a, kc, mc):
    if kind == "BJ":
        return 4 * a + 2 * kc + mc          # j = 0..7  -> 0..31
    if kind == "A8P":
        return 32 + 4 * a + 2 * kc + mc     # r = 0..2  -> 32..43
    if kind == "A32":
        return 44 + 2 * kc + mc
    if kind == "A1":
        return 48 + 2 * kc + mc
    if kind == "W1":
        return 52 + 2 * kc + mc
    raise KeyError(kind)


def _host_pack(weight_f32):
    """Pack all lhsT 128x128 tiles into one (NT,128,128) float32 array.

    matmul(out, lhsT, rhs) computes lhsT.T @ rhs, so for out = Mat @ v the
    (kc, mc) tile is Mat.T[128kc:128(kc+1), 128mc:128(mc+1)].
    """
    A = weight_f32[:, :, 0].astype(np.float64)
    W1 = weight_f32[:, :, 1].astype(np.float64)

    def tiles(mat):
        mt = mat.T.astype(np.float32)
        return [mt[128 * kc:128 * (kc + 1), 128 * mc:128 * (mc + 1)]
                for kc in range(2) for mc in range(2)]

    Apow = [np.linalg.matrix_power(A, p) for p in range(9)]
    pack = []
    for j in range(8):
        pack += tiles(Apow[7 - j] @ W1)                    # BJ
    for r in range(3):
        pack += tiles(np.linalg.matrix_power(A, 8 * (3 - r)))  # A8P: A24,A16,A8
    pack += tiles(np.linalg.matrix_power(A, 32))           # A32
    pack += tiles(A)                                       # A1
    pack += tiles(W1)                                      # W1
    assert len(pack) == NT
    return np.stack(pack, axis=0)


def _build_program():
    """Build + bacc-compile the per-core Bass program. Returns nc."""
    import concourse.bacc as bacc
    import concourse.tile as tile
    from concourse import mybir

    f32r = mybir.dt.float32r
    f32 = mybir.dt.float32
    bf16 = mybir.dt.bfloat16

    nc = bacc.Bacc("TRN2", target_bir_lowering=False, debug=False,
                   num_devices=N_CORES)
    x_in = nc.dram_tensor("x", [B_LOC, C, N], f32r, kind="ExternalInput").ap()
    wp_in = nc.dram_tensor("wpack", [NT, 128, 128], f32r,
                           kind="ExternalInput").ap()
    out = nc.dram_tensor("out", [B_LOC, C, N], f32r, kind="ExternalOutput").ap()

    with tile.TileContext(nc) as tc:
        import contextlib
        with contextlib.ExitStack() as ctx:
            wpool = ctx.enter_context(tc.tile_pool(name="wpool", bufs=1))
            xpool = ctx.enter_context(tc.tile_pool(name="xpool", bufs=1))
            spool = ctx.enter_context(tc.tile_pool(name="spool", bufs=1))
            pspool = ctx.enter_context(tc.tile_pool(name="ps", bufs=8,
                                                    space="PSUM"))

            # ---- persistent tiles ----
            wpk = wpool.tile([128, NT * 128], f32r, tag="wpk", name="wpk")
            wt = lambda kind, a, kc, mc: wpk[:, 128 * _wi(kind, a, kc, mc):
                                             128 * (_wi(kind, a, kc, mc) + 1)]
            # two extra leading columns per batch element: x[256+t] lives at
            # col t+2 and s[v] lands at col v (s[8g+i] overwrites the x slot
            # one LEFT of the one its step consumes). Col 0 holds s0, so the
            # unit-0 output DMA covers out col 0 too.
            TC = T + 2
            xres = [xpool.tile([128, B_LOC * TC], f32r, tag=f"x{kc}",
                               name=f"x{kc}") for kc in range(2)]
            l1 = [spool.tile([128, B_LOC * G], f32r, tag=f"l1_{mc}",
                             name=f"l1_{mc}") for mc in range(2)]
            u1 = [spool.tile([128, B_LOC * G], f32r, tag=f"u1_{mc}",
                             name=f"u1_{mc}") for mc in range(2)]
            zz = [spool.tile([128, B_LOC * (H + 2)], f32r, tag=f"zz_{mc}",
                             name=f"zz_{mc}") for mc in range(2)]

            xr3 = [xres[kc].rearrange("p (b t) -> p b t", b=B_LOC)
                   for kc in range(2)]  # col t+2 holds x[:, 256+t]; col v ends up s[v]
            l13 = [l1[mc].rearrange("p (b g) -> p b g", b=B_LOC)
                   for mc in range(2)]
            u13 = [u1[mc].rearrange("p (b g) -> p b g", b=B_LOC)
                   for mc in range(2)]
            zz3 = [zz[mc].rearrange("p (b k) -> p b k", b=B_LOC)
                   for mc in range(2)]

            def st(ap, start, count, step):
                # strided slice with exact end bound (strict AP indexing)
                return ap[:, :, start:start + step * (count - 1) + 1:step]

            def stb(ap, b, start, count, step):
                # per-batch-element strided slice
                return ap[:, b, start:start + step * (count - 1) + 1:step]

            # round-robin PSUM->SBUF copy engines (only DVE + ACT see PSUM)
            _cp = [0]

            def copy_ps(dst, src):
                if _cp[0] % 2 == 0:
                    nc.vector.tensor_copy(dst, src)
                else:
                    nc.scalar.copy(dst, src)
                _cp[0] += 1

            # ---- phase 0 DMAs (the rest are emitted interleaved with
            # compute below -- emission order IS the DMA queue order) ----
            # BJ tiles first so up1 can start as soon as x unit 0 lands.
            nc.sync.dma_start(
                wpk[:, 0:32 * 128].rearrange("p (t f) -> p t f", t=32),
                wp_in[0:32].rearrange("t p f -> p t f"))

            def dma_x_unit(b, u):
                # per-(b,kc) transfers keep write intervals local, so
                # interval-granular dep tracking stays precise
                g0, gw = UNITS[u]
                t0, tw = 8 * g0, 8 * gw
                for kc in range(2):
                    nc.sync.dma_start(
                        xr3[kc][:, b, 2 + t0:2 + t0 + tw],
                        x_in[b, 128 * kc:128 * (kc + 1),
                             N_DIL + t0:N_DIL + t0 + tw])

            def dma_rest_weights():
                nc.sync.dma_start(
                    wpk[:, 32 * 128:].rearrange("p (t f) -> p t f", t=NT - 32),
                    wp_in[32:].rearrange("t p f -> p t f"))

            def dma_small():
                # s0 -> xres col 0 (rides the unit-0 output DMA from there);
                # zz col 1 (seam) filled from it on gpsimd, zz col 0 zeroed
                # as s0*0 (memset on f32r is invalid ISA).
                for mc in range(2):
                    for b in range(B_LOC):
                        nc.sync.dma_start(xr3[mc][:, b, 0:1],
                                          x_in[b, 128 * mc:128 * (mc + 1),
                                               0:1])
                        nc.gpsimd.tensor_copy(zz3[mc][:, b, 1:2],
                                              xr3[mc][:, b, 0:1])
                        nc.vector.tensor_scalar_mul(zz3[mc][:, b, 0:1],
                                                    zz3[mc][:, b, 1:2], 0.0)

            def dma_tail():
                # untouched tail: from the already-resident x (cols
                # 7682..7938); down1 unit 2's overwrites are WAR-ordered
                # after this read, so this must sit EARLY in the queue.
                for b in range(B_LOC):
                    for kc in range(2):
                        nc.sync.dma_start(
                            out[b, 128 * kc:128 * (kc + 1), T:N],
                            xr3[kc][:, b, 2 + T - N_DIL:2 + T])

            # ---- up1(b,u): l1[g] = sum_j (A^(7-j) W1) x[8g+j+256] ----
            def up1(b, u):
                g0, gw = UNITS[u]
                for mc in range(2):
                    ps = pspool.tile([128, gw], f32, tag="ps", name="ps")
                    for j in range(8):
                        for kc in range(2):
                            rhs = stb(xr3[kc], b, 2 + 8 * g0 + j, gw, 8)
                            nc.tensor.matmul(ps[:], wt("BJ", j, kc, mc), rhs,
                                             start=(j == 0 and kc == 0),
                                             stop=(j == 7 and kc == 1))
                    copy_ps(l13[mc][:, b, g0:g0 + gw], ps[:])

            # ---- level 2 chunk: 32-block sums, seam, 3-step down scan ----
            def level2(c):
                h0, hw = L2CHUNKS[c]
                # up2: l2[h] -> zz[h+2]; the r=3 (identity) term rides the
                # PSUM->SBUF transfer as a DVE add.
                for mc in range(2):
                    ps = pspool.tile([128, B_LOC * hw], f32, tag="ps",
                                     name="ps")
                    for r in range(3):
                        for kc in range(2):
                            rhs = st(l13[kc], 4 * h0 + r, hw, 4)
                            nc.tensor.matmul(ps[:], wt("A8P", r, kc, mc), rhs,
                                             start=(r == 0 and kc == 0),
                                             stop=(r == 2 and kc == 1))
                    nc.vector.tensor_add(
                        zz3[mc][:, :, h0 + 2:h0 + 2 + hw], ps[:],
                        st(l13[mc], 4 * h0 + 3, hw, 4))
                # seam: u1[4h] = s[32h] = zz[h+1] + A32 zz[h]
                for mc in range(2):
                    ps = pspool.tile([128, B_LOC * hw], f32, tag="ps",
                                     name="ps")
                    for kc in range(2):
                        nc.tensor.matmul(ps[:], wt("A32", 0, kc, mc),
                                         zz3[kc][:, :, h0:h0 + hw],
                                         start=(kc == 0), stop=(kc == 1))
                    nc.vector.tensor_add(
                        st(u13[mc], 4 * h0, hw, 4), ps[:],
                        zz3[mc][:, :, h0 + 1:h0 + 1 + hw])
                # down2: u1[4h+r] = A8 u1[4h+r-1] + l1[4h+r-1]
                for r in range(1, 4):
                    for mc in range(2):
                        ps = pspool.tile([128, B_LOC * hw], f32, tag="ps",
                                         name="ps")
                        for kc in range(2):
                            rhs = st(u13[kc], 4 * h0 + r - 1, hw, 4)
                            nc.tensor.matmul(ps[:], wt("A8P", 2, kc, mc), rhs,
                                             start=(kc == 0), stop=(kc == 1))
                        nc.vector.tensor_add(
                            st(u13[mc], 4 * h0 + r, hw, 4), ps[:],
                            st(l13[mc], 4 * h0 + r - 1, hw, 4))

            # ---- down1(u): s[8g+i] = A s[8g+i-1] + W1 x[8g+i+255];
            # s[v] overwrites xres col v-1 (the just-consumed x slot), so the
            # output DMA is a contiguous slice of xres. ----
            def down1(b, u):
                g0, gw = UNITS[u]
                for i in range(1, 8):
                    for mc in range(2):
                        ps = pspool.tile([128, gw], f32, tag="ps", name="ps")
                        for kc in range(2):
                            # W1 term reads x t=8g+i-1 at col 8g+i+1
                            nc.tensor.matmul(
                                ps[:], wt("W1", 0, kc, mc),
                                stb(xr3[kc], b, 8 * g0 + i + 1, gw, 8),
                                start=(kc == 0), stop=False)
                        for kc in range(2):
                            # A term reads s[8g+i-1] at col 8g+i-1
                            if i == 1:
                                rhs = u13[kc][:, b, g0:g0 + gw]
                            else:
                                rhs = stb(xr3[kc], b, 8 * g0 + i - 1, gw, 8)
                            nc.tensor.matmul(ps[:], wt("A1", 0, kc, mc), rhs,
                                             start=False, stop=(kc == 1))
                        # s[8g+i] -> col 8g+i: the x slot the PREVIOUS step
                        # consumed; never a read operand of this step.
                        copy_ps(stb(xr3[mc], b, 8 * g0 + i, gw, 8), ps[:])
                # residues: s[8g] = u1[g] -> col 8g, dead after step 7's
                # W1 read of the preceding block (this/previous unit).
                gr0 = max(g0, 1)
                for mc in range(2):
                    nc.gpsimd.tensor_copy(
                        stb(xr3[mc], b, 8 * gr0, g0 + gw - gr0, 8),
                        u13[mc][:, b, gr0:g0 + gw])
                # this unit's out cols are final: stream them to DRAM
                # (s[v] sits at col v; col 0 = s0 covers out col 0 for u0)
                v0 = 8 * g0
                v1 = 8 * (g0 + gw)
                for mc in range(2):
                    nc.sync.dma_start(
                        out[b, 128 * mc:128 * (mc + 1), v0:v1],
                        xr3[mc][:, b, v0:v1])

            # ---- pipeline: all x-in DMAs front-loaded densely (the queue
            # then drains queued output units with no idle), compute
            # interleaved by readiness ----
            dma_x_unit(0, 0)
            dma_x_unit(1, 0)
            dma_rest_weights()
            dma_small()
            up1(0, 0)
            up1(1, 0)
            level2(0)
            dma_x_unit(0, 1)
            dma_x_unit(1, 1)
            dma_x_unit(0, 2)
            dma_x_unit(1, 2)
            dma_tail()
            down1(0, 0)            # emits out(b0,u0)
            down1(1, 0)
            up1(0, 1)
            up1(1, 1)
            level2(1)
            down1(0, 1)
            down1(1, 1)
            up1(0, 2)
            up1(1, 2)
            level2(2)
            down1(0, 2)
            down1(1, 2)

    nc.compile()
    return nc


_CACHE = {}


def _get_program():
    if "nc" not in _CACHE:
        _CACHE["nc"] = _build_program()
    return _CACHE["nc"]


LAST_RESULTS = None  # test harness reads exec_time_ns off this


def kernel(x, weight, n_dil):
    import os
    from concourse.bass_utils import run_bass_kernel_spmd
    global LAST_RESULTS

    x = np.asarray(x)
    weight = np.asarray(weight)
    assert int(n_dil) == N_DIL and x.shape == (B_FULL, C, N)
    nc = _get_program()
    wpack = _host_pack(weight.astype(np.float32))

    xs = x.astype(np.float32).reshape(N_CORES, B_LOC, C, N)
    in_maps = [{"x": xs[i], "wpack": wpack} for i in range(N_CORES)]
    trace = bool(os.environ.get("KERNEL_TRACE"))
    res = run_bass_kernel_spmd(nc, in_maps, list(range(N_CORES)), trace=trace)
    LAST_RESULTS = res
    out = np.concatenate([res.results[i]["out"] for i in range(N_CORES)],
                         axis=0)
    return out.astype(x.dtype, copy=False)
